# Initial kernel scaffold
#
"""MiniGRU Trainium2 kernel.

Problem: h_t = (1-z_t) h_{t-1} + z_t g(p_t), with
  z_t = sigmoid(x_t @ Wz^T + bz), p_t = x_t @ Wh^T + bh,
  g(x) = x + 0.5 for x>=0 else sigmoid(x)  (note g(x) = max(x+0.5, sigmoid(x))),
  initial state g(h_0).  Shapes: x [4, 4096, 1024], H = 1024.

Sharding: 8 cores = batch(4) x H-halves(2). No collectives. Each core gets
host-pre-transposed inputs:
  xT  [1024 din, 4096 seq]   (moving operand for both GEMMs)
  wzT/whT [1024 din, 512 ch] (stationary operands)
  aux [128, 5, 4]            per chan-group columns: g(h0), bz, -bz, bh, bh+0.5
and returns hT [512 ch, 4096 seq]; host transposes back.

Device dataflow per (seq-block of 512, chan-group of 128):
  PE: 8+8 accumulating fp32r matmuls -> PSUM kz, kh  [128 ch, 512 seq]
  ACT: a = sigmoid(-kz-bz), z = sigmoid(kz+bz), sp = sigmoid(kh+bh)
  DVE: gp = max(kh+(bh+0.5), sp); b = z*gp;
       h = tensor_tensor_scan(a, b, init)  -- state = a*state + b along seq
Scan state chains across seq-blocks via initial=prev_h[:, -1:].
"""

import numpy as np

import concourse.bass as bass
import concourse.bacc as bacc
import concourse.mybir as mybir
import concourse.tile as tile
from concourse.bass_utils import run_bass_kernel_spmd

F32 = mybir.dt.float32
F32R = mybir.dt.float32r
AF = mybir.ActivationFunctionType
ALU = mybir.AluOpType

BS, SEQ, DIN, H = 4, 4096, 1024, 1024
NCORES = 8
H_SPLIT = 2
CH = H // H_SPLIT  # channels per core


def build_nc(seq=SEQ, din=DIN, ch=CH, nb=512, x_bufs=3):
    """Build the single-core SPMD Bass program."""
    kt = din // 128   # contraction tiles
    mg = ch // 128    # chan groups
    nblk = seq // nb  # seq blocks

    nc = bacc.Bacc("TRN2", target_bir_lowering=False, debug=False)

    xT_d = nc.dram_tensor("xT", [din, seq], F32, kind="ExternalInput")
    wzT_d = nc.dram_tensor("wzT", [din, ch], F32, kind="ExternalInput")
    whT_d = nc.dram_tensor("whT", [din, ch], F32, kind="ExternalInput")
    aux_d = nc.dram_tensor("aux", [128, 5, mg], F32, kind="ExternalInput")
    hT_d = nc.dram_tensor("hT", [ch, seq], F32, kind="ExternalOutput")

    xT_r = xT_d.ap().rearrange("(k p) s -> p k s", p=128)
    wzT_r = wzT_d.ap().rearrange("(k p) c -> p k c", p=128)
    whT_r = whT_d.ap().rearrange("(k p) c -> p k c", p=128)

    with tile.TileContext(nc) as tc:
        with (
            tc.tile_pool(name="wpool", bufs=1) as wpool,
            tc.tile_pool(name="xpool", bufs=x_bufs) as xpool,
            tc.tile_pool(name="epool", bufs=2) as epool,
            tc.tile_pool(name="hpool", bufs=1) as hpool,
            tc.tile_pool(name="psum", bufs=4, space="PSUM") as psum,
        ):
            wz_sb = wpool.tile([128, kt, ch], F32)
            wh_sb = wpool.tile([128, kt, ch], F32)
            aux_sb = wpool.tile([128, 5, mg], F32)
            nc.sync.dma_start(wz_sb[:], wzT_r)
            nc.sync.dma_start(wh_sb[:], whT_r)
            nc.sync.dma_start(aux_sb[:], aux_d.ap())

            # per chan-group scan-state chain: AP of [128, 1]
            h_prev = [aux_sb[:, 0, m : m + 1] for m in range(mg)]

            for blk in range(nblk):
                xb = xpool.tile([128, kt, nb], F32, tag="xb")
                nc.sync.dma_start(xb[:], xT_r[:, :, blk * nb : (blk + 1) * nb])

                for m in range(mg):
                    ms = slice(m * 128, (m + 1) * 128)
                    kz = psum.tile([128, nb], F32, tag="kz")
                    kh = psum.tile([128, nb], F32, tag="kh")
                    for k in range(kt):
                        nc.tensor.matmul(
                            kz[:],
                            wz_sb[:, k, ms].bitcast(F32R),
                            xb[:, k, :].bitcast(F32R),
                            start=(k == 0),
                            stop=(k == kt - 1),
                        )
                    for k in range(kt):
                        nc.tensor.matmul(
                            kh[:],
                            wh_sb[:, k, ms].bitcast(F32R),
                            xb[:, k, :].bitcast(F32R),
                            start=(k == 0),
                            stop=(k == kt - 1),
                        )

                    a_t = epool.tile([128, nb], F32, tag="a")
                    z_t = epool.tile([128, nb], F32, tag="z")
                    sp_t = epool.tile([128, nb], F32, tag="sp")
                    gp_t = epool.tile([128, nb], F32, tag="gp")
                    b_t = epool.tile([128, nb], F32, tag="b")
                    h_t = hpool.tile([128, nb], F32, tag=f"h{m}", bufs=3)

                    # a = sigmoid(-(kz + bz));  z = sigmoid(kz + bz)
                    nc.scalar.activation(
                        a_t[:], kz[:], AF.Sigmoid,
                        bias=aux_sb[:, 2, m : m + 1], scale=-1.0,
                    )
                    nc.scalar.activation(
                        z_t[:], kz[:], AF.Sigmoid,
                        bias=aux_sb[:, 1, m : m + 1], scale=1.0,
                    )
                    # sp = sigmoid(kh + bh)
                    nc.scalar.activation(
                        sp_t[:], kh[:], AF.Sigmoid,
                        bias=aux_sb[:, 3, m : m + 1], scale=1.0,
                    )
                    # gp = max(kh + (bh+0.5), sp)
                    nc.vector.scalar_tensor_tensor(
                        gp_t[:], kh[:], aux_sb[:, 4, m : m + 1], sp_t[:],
                        op0=ALU.add, op1=ALU.max,
                    )
                    # b = z * gp
                    nc.vector.tensor_mul(b_t[:], z_t[:], gp_t[:])
                    # h scan: state = a*state + b
                    nc.vector.tensor_tensor_scan(
                        h_t[:], a_t[:], b_t[:], h_prev[m],
                        op0=ALU.mult, op1=ALU.add,
                    )
                    h_prev[m] = h_t[:, nb - 1 : nb]

                    nc.sync.dma_start(
                        hT_d.ap()[ms, blk * nb : (blk + 1) * nb], h_t[:]
                    )

    nc.compile()
    return nc


def _g(x):
    return np.where(x >= 0, x + 0.5, 1.0 / (1.0 + np.exp(-x)))


def make_in_maps(x, h_0, Wz, bz, Wh, bh, seq=SEQ, din=DIN, ch=CH):
    """Host-side shard: returns one in_map per core."""
    mg = ch // 128
    gh0 = _g(h_0.astype(np.float32))  # [bs, 1, H]
    in_maps = []
    for c in range(NCORES):
        b, g = divmod(c, H_SPLIT)
        cs = slice(g * ch, (g + 1) * ch)
        aux = np.zeros((128, 5, mg), dtype=np.float32)
        aux[:, 0, :] = gh0[b, 0, cs].reshape(mg, 128).T
        aux[:, 1, :] = bz[cs].reshape(mg, 128).T
        aux[:, 2, :] = -bz[cs].reshape(mg, 128).T
        aux[:, 3, :] = bh[cs].reshape(mg, 128).T
        aux[:, 4, :] = (bh[cs] + 0.5).reshape(mg, 128).T
        in_maps.append(
            {
                "xT": np.ascontiguousarray(x[b].T.astype(np.float32)),
                "wzT": np.ascontiguousarray(Wz[cs, :].T.astype(np.float32)),
                "whT": np.ascontiguousarray(Wh[cs, :].T.astype(np.float32)),
                "aux": aux,
            }
        )
    return in_maps


_NC_CACHE = {}


def get_nc():
    if "nc" not in _NC_CACHE:
        _NC_CACHE["nc"] = build_nc()
    return _NC_CACHE["nc"]


def kernel(x, h_0, Wz, bz, Wh, bh, trace=False, trace_kwargs=None):
    x = np.asarray(x)
    h_0 = np.asarray(h_0)
    Wz = np.asarray(Wz)
    bz = np.asarray(bz)
    Wh = np.asarray(Wh)
    bh = np.asarray(bh)

    nc = get_nc()
    in_maps = make_in_maps(x, h_0, Wz, bz, Wh, bh)
    res = run_bass_kernel_spmd(
        nc, in_maps, core_ids=list(range(NCORES)),
        trace=trace, **(trace_kwargs or {}),
    )
    out = np.empty((BS, SEQ, H), dtype=np.float32)
    for c in range(NCORES):
        b, g = divmod(c, H_SPLIT)
        out[b, :, g * CH : (g + 1) * CH] = res.results[c]["hT"].T
    if trace:
        kernel.last_result = res
    return out


# revision 5
# speedup vs baseline: 1.7434x; 1.7434x over previous
"""MiniGRU Trainium2 kernel.

Problem: h_t = (1-z_t) h_{t-1} + z_t g(p_t), with
  z_t = sigmoid(x_t @ Wz^T + bz), p_t = x_t @ Wh^T + bh,
  g(x) = x + 0.5 for x>=0 else sigmoid(x)  (note g(x) = max(x+0.5, sigmoid(x))),
  initial state g(h_0).  Shapes: x [4, 4096, 1024], H = 1024.

Sharding: 8 cores = batch(4) x H-halves(2). No collectives. Each core gets
host-pre-transposed inputs:
  xT  [1024 din, 4096 seq]   (moving operand for both GEMMs)
  wzT/whT [1024 din, 512 ch] (stationary operands)
  aux [128, 5, 4]            per chan-group columns: g(h0), bz, -bz, bh, bh+0.5
and returns hT [512 ch, 4096 seq]; host transposes back.

Device dataflow per (seq-block of 512, chan-group of 128):
  PE: 8+8 accumulating fp32r matmuls -> PSUM kz, kh  [128 ch, 512 seq]
  ACT: a = sigmoid(-kz-bz), z = sigmoid(kz+bz), sp = sigmoid(kh+bh)
  DVE: gp = max(kh+(bh+0.5), sp); b = z*gp;
       h = tensor_tensor_scan(a, b, init)  -- state = a*state + b along seq
Scan state chains across seq-blocks via initial=prev_h[:, -1:].
"""

import numpy as np

import concourse.bass as bass
import concourse.bacc as bacc
import concourse.mybir as mybir
import concourse.tile as tile
from concourse.bass_utils import run_bass_kernel_spmd

F32 = mybir.dt.float32
F32R = mybir.dt.float32r
AF = mybir.ActivationFunctionType
ALU = mybir.AluOpType

BS, SEQ, DIN, H = 4, 4096, 1024, 1024
NCORES = 8
H_SPLIT = 2
CH = H // H_SPLIT  # channels per core


def build_nc(seq=SEQ, din=DIN, ch=CH, nb=512, x_bufs=3, loop_reps=1):
    """Build the single-core SPMD Bass program.

    loop_reps > 1 wraps the whole body in a hardware For_i loop that
    recomputes the same output N times — used only for benchmarking
    (slope of wall time vs reps isolates HW exec time from RPC overhead).
    """
    kt = din // 128   # contraction tiles
    mg = ch // 128    # chan groups
    nblk = seq // nb  # seq blocks

    nc = bacc.Bacc("TRN2", target_bir_lowering=False, debug=False)

    xT_d = nc.dram_tensor("xT", [din, seq], F32R, kind="ExternalInput")
    wzT_d = nc.dram_tensor("wzT", [din, ch], F32R, kind="ExternalInput")
    whT_d = nc.dram_tensor("whT", [din, ch], F32R, kind="ExternalInput")
    aux_d = nc.dram_tensor("aux", [128, 5, mg], F32, kind="ExternalInput")
    hT_d = nc.dram_tensor("hT", [ch, seq], F32, kind="ExternalOutput")

    xT_r = xT_d.ap().rearrange("(k p) s -> p k s", p=128)
    wzT_r = wzT_d.ap().rearrange("(k p) c -> p k c", p=128)
    whT_r = whT_d.ap().rearrange("(k p) c -> p k c", p=128)

    with tile.TileContext(nc) as tc:
        with (
            tc.tile_pool(name="wpool", bufs=1) as wpool,
            tc.tile_pool(name="xpool", bufs=x_bufs) as xpool,
            tc.tile_pool(name="epool", bufs=2) as epool,
            tc.tile_pool(name="hpool", bufs=1) as hpool,
            tc.tile_pool(name="psum", bufs=4, space="PSUM") as psum,
        ):
            wz_sb = wpool.tile([128, kt, ch], F32R)
            wh_sb = wpool.tile([128, kt, ch], F32R)
            aux_sb = wpool.tile([128, 5, mg], F32)
            nc.sync.dma_start(wz_sb[:], wzT_r)
            nc.sync.dma_start(wh_sb[:], whT_r)
            nc.sync.dma_start(aux_sb[:], aux_d.ap())

            def emit_body():
                # per chan-group scan-state chain: AP of [128, 1]
                h_prev = [aux_sb[:, 0, m : m + 1] for m in range(mg)]
                for blk in range(nblk):
                    xb = xpool.tile([128, kt, nb], F32R, tag="xb", name="xb")
                    nc.sync.dma_start(xb[:], xT_r[:, :, blk * nb : (blk + 1) * nb])

                    for m in range(mg):
                        ms = slice(m * 128, (m + 1) * 128)
                        kz = psum.tile([128, nb], F32, tag="kz", name="kz")
                        kh = psum.tile([128, nb], F32, tag="kh", name="kh")
                        for k in range(kt):
                            nc.tensor.matmul(
                                kz[:], wz_sb[:, k, ms], xb[:, k, :],
                                start=(k == 0), stop=(k == kt - 1),
                            )
                        for k in range(kt):
                            nc.tensor.matmul(
                                kh[:], wh_sb[:, k, ms], xb[:, k, :],
                                start=(k == 0), stop=(k == kt - 1),
                            )

                        a_t = epool.tile([128, nb], F32, tag="a", name="a_t")
                        z_t = epool.tile([128, nb], F32, tag="z", name="z_t")
                        sp_t = epool.tile([128, nb], F32, tag="sp", name="sp_t")
                        gp_t = epool.tile([128, nb], F32, tag="gp", name="gp_t")
                        b_t = epool.tile([128, nb], F32, tag="b", name="b_t")
                        h_t = hpool.tile([128, nb], F32, tag=f"h{m}", bufs=3, name="h_t")

                        # a = sigmoid(-(kz + bz));  z = sigmoid(kz + bz)
                        nc.scalar.activation(
                            a_t[:], kz[:], AF.Sigmoid,
                            bias=aux_sb[:, 2, m : m + 1], scale=-1.0,
                        )
                        nc.scalar.activation(
                            z_t[:], kz[:], AF.Sigmoid,
                            bias=aux_sb[:, 1, m : m + 1], scale=1.0,
                        )
                        # sp = sigmoid(kh + bh)
                        nc.scalar.activation(
                            sp_t[:], kh[:], AF.Sigmoid,
                            bias=aux_sb[:, 3, m : m + 1], scale=1.0,
                        )
                        # gp = max(kh + (bh+0.5), sp)
                        nc.vector.scalar_tensor_tensor(
                            gp_t[:], kh[:], aux_sb[:, 4, m : m + 1], sp_t[:],
                            op0=ALU.add, op1=ALU.max,
                        )
                        # b = z * gp
                        nc.vector.tensor_mul(b_t[:], z_t[:], gp_t[:])
                        # h scan: state = a*state + b
                        nc.vector.tensor_tensor_scan(
                            h_t[:], a_t[:], b_t[:], h_prev[m],
                            op0=ALU.mult, op1=ALU.add,
                        )
                        h_prev[m] = h_t[:, nb - 1 : nb]

                        nc.sync.dma_start(
                            hT_d.ap()[ms, blk * nb : (blk + 1) * nb], h_t[:]
                        )

            if loop_reps == 1:
                emit_body()
            else:
                with tc.For_i(0, loop_reps, 1):
                    emit_body()

    nc.compile()
    return nc


def _g(x):
    return np.where(x >= 0, x + 0.5, 1.0 / (1.0 + np.exp(-x)))


def make_in_maps(x, h_0, Wz, bz, Wh, bh, seq=SEQ, din=DIN, ch=CH):
    """Host-side shard: returns one in_map per core."""
    mg = ch // 128
    gh0 = _g(h_0.astype(np.float32))  # [bs, 1, H]
    in_maps = []
    for c in range(NCORES):
        b, g = divmod(c, H_SPLIT)
        cs = slice(g * ch, (g + 1) * ch)
        aux = np.zeros((128, 5, mg), dtype=np.float32)
        aux[:, 0, :] = gh0[b, 0, cs].reshape(mg, 128).T
        aux[:, 1, :] = bz[cs].reshape(mg, 128).T
        aux[:, 2, :] = -bz[cs].reshape(mg, 128).T
        aux[:, 3, :] = bh[cs].reshape(mg, 128).T
        aux[:, 4, :] = (bh[cs] + 0.5).reshape(mg, 128).T
        in_maps.append(
            {
                "xT": np.ascontiguousarray(x[b].T.astype(np.float32)),
                "wzT": np.ascontiguousarray(Wz[cs, :].T.astype(np.float32)),
                "whT": np.ascontiguousarray(Wh[cs, :].T.astype(np.float32)),
                "aux": aux,
            }
        )
    return in_maps


_NC_CACHE = {}


def get_nc():
    if "nc" not in _NC_CACHE:
        _NC_CACHE["nc"] = build_nc()
    return _NC_CACHE["nc"]


def kernel(x, h_0, Wz, bz, Wh, bh, trace=False, trace_kwargs=None):
    x = np.asarray(x)
    h_0 = np.asarray(h_0)
    Wz = np.asarray(Wz)
    bz = np.asarray(bz)
    Wh = np.asarray(Wh)
    bh = np.asarray(bh)

    nc = get_nc()
    in_maps = make_in_maps(x, h_0, Wz, bz, Wh, bh)
    res = run_bass_kernel_spmd(
        nc, in_maps, core_ids=list(range(NCORES)),
        trace=trace, **(trace_kwargs or {}),
    )
    out = np.empty((BS, SEQ, H), dtype=np.float32)
    for c in range(NCORES):
        b, g = divmod(c, H_SPLIT)
        out[b, :, g * CH : (g + 1) * CH] = res.results[c]["hT"].T
    if trace:
        kernel.last_result = res
    return out
